# revision 6
# baseline (speedup 1.0000x reference)
"""Epipolar attention kernel for Trainium2 (8 NeuronCores, batch-parallel).

Math notes (derived from the reference):
  - f_tar is dead code: the output only depends on f_src / K1 / K2 / R / t.
  - With x0=0, x1=W the distance field factorizes rank-3:
        d[b,i,j] = |px_i*alpha[b,j] + py_i*beta[b,j] + gamma[b,j]|
    where alpha = dy/L, beta = -dx/L, gamma = y0*dx/L, L = sqrt(dx^2+dy^2).
  - softmax_j(5*(d-0.1)) == softmax_j(5*d)           (shift invariance)
  - softmax_i(1 - p)     == softmax_i(-p), and p in (0,1] means exp(-p) needs
    no max subtraction.
The 3x3 SVD / inverse chain (B=16) plus the rank-3 coefficient prep is O(B*HW)
host work; all O(B*HW^2) work runs on the NeuronCores.

v2 performance structure:
  - Stage-1 S = P^T Q runs as K=6 matmuls (hi+lo bf16 rows merged) with 4-way
    PE row tiling (tile_position) so 4 matmuls stream concurrently.
  - |S| row-max comes straight off PSUM via reduce_max(apply_absolute_value);
    the elementwise |S| pass is split between DVE (scalar_tensor_tensor) and
    ACT (Abs) to balance engine load; exp folds the x5 scale.
  - diag(1/s1) prep, the x5 bias scale, and the fw = fa/s2 row scaling run on
    the otherwise-idle GpSimd engine.
  - The output GEMM partially rides along with stage-2: 2 of 8 i-tiles
    accumulate in dedicated PSUM banks as each e2 j-tile is produced, the
    remaining 6 run from SBUF afterwards, overlapping the next batch.
"""

import numpy as np
import ml_dtypes

import concourse.bass as bass
import concourse.bacc as bacc
import concourse.tile as tile
import concourse.mybir as mybir
from concourse.bass_utils import run_bass_kernel_spmd

B, C, H, W = 16, 512, 32, 32
HW = H * W          # 1024
NCORES = 8
BPC = B // NCORES   # batches per core
NT = HW // 128      # 128-row tiles per HW dim
F32 = mybir.dt.float32
BF16 = mybir.dt.bfloat16
AF = mybir.ActivationFunctionType
AX = mybir.AxisListType
ALU = mybir.AluOpType

# i-tiles that ride along with stage-2 (accumulate per produced e2 j-tile)
N_RIDE = 2


# ---------------------------------------------------------------- host math
def _line_coeffs(K1, K2, R, t):
    """Float32 numpy mirror of the reference's per-batch line geometry.

    Returns Q (B, 3, HW) with rows [alpha, beta, gamma] and P (3, HW) with
    rows [px, py, 1].
    """
    K1 = np.asarray(K1, np.float32)
    K2 = np.asarray(K2, np.float32)
    R = np.asarray(R, np.float32)
    t = np.asarray(t, np.float32)

    z = np.zeros_like(t[:, 0])
    tx, ty, tz = t[:, 0], t[:, 1], t[:, 2]
    skew = np.stack(
        [
            np.stack([z, -tz, ty], axis=-1),
            np.stack([tz, z, -tx], axis=-1),
            np.stack([-ty, tx, z], axis=-1),
        ],
        axis=1,
    )
    E = skew @ R
    U, S, Vt = np.linalg.svd(E)
    S = S * np.array([1.0, 1.0, 0.0], dtype=S.dtype)
    E = U @ (S[:, :, None] * Vt)
    Fm = np.linalg.inv(np.swapaxes(K2, 1, 2)) @ E @ np.linalg.inv(K1)
    Fm = Fm.astype(np.float32)

    ix, iy = np.meshgrid(
        np.arange(H, dtype=np.float32), np.arange(W, dtype=np.float32), indexing="ij"
    )
    px = ix.reshape(-1)
    py = iy.reshape(-1)
    idx = np.stack([px, py, np.ones_like(px)], axis=0)  # (3, HW)

    lines = Fm @ idx[None]  # (B, 3, HW)
    a, b, c = lines[:, 0], lines[:, 1], lines[:, 2]
    x0 = np.zeros_like(a)
    y0 = -c / b
    x1 = np.full_like(a, float(W))
    y1 = -(c + a * float(W)) / b
    dx = x0 - x1
    dy = y0 - y1
    L = np.sqrt(dx * dx + dy * dy)

    alpha = dy / L
    beta = -dx / L
    gamma = (y0 * dx) / L
    Q = np.stack([alpha, beta, gamma], axis=1).astype(np.float32)  # (B, 3, HW)
    P = idx.astype(np.float32)
    return Q, P


# ---------------------------------------------------------------- device IR
def _build_nc():
    nc = bacc.Bacc("TRN2", target_bir_lowering=False, debug=False)

    # P6/Q6 carry [hi; lo] bf16 splits stacked to K=6, replicated at partition
    # offsets 0/32/64/96 so 4-way PE row tiling can run 4 matmuls at once.
    pmat_d = nc.dram_tensor("pmat", [128, HW], BF16, kind="ExternalInput")
    qmat_d = nc.dram_tensor("qmat", [BPC, 128, HW], BF16, kind="ExternalInput")
    fsrc_d = nc.dram_tensor("fsrc", [BPC, HW, C], BF16, kind="ExternalInput")
    ident_d = nc.dram_tensor("ident", [128, 128], BF16, kind="ExternalInput")
    out_d = nc.dram_tensor("out", [BPC, HW, C], F32, kind="ExternalOutput")

    with tile.TileContext(nc) as tc:
        with (
            tc.tile_pool(name="const", bufs=1) as const,
            tc.tile_pool(name="q", bufs=2) as qpool,
            tc.tile_pool(name="f", bufs=2) as fpool,
            tc.tile_pool(name="z", bufs=3) as zpool,
            tc.tile_pool(name="e", bufs=2) as epool,
            tc.tile_pool(name="dg", bufs=2) as dgpool,
            tc.tile_pool(name="e2", bufs=2) as e2pool,
            tc.tile_pool(name="stat", bufs=2) as stat,
            tc.tile_pool(name="o", bufs=4) as opool,
            tc.tile_pool(name="sps", bufs=2, space="PSUM") as spspool,
            tc.tile_pool(name="tp", bufs=1, space="PSUM") as tppool,
            tc.tile_pool(name="g", bufs=1, space="PSUM") as gpool,
        ):
            pm = const.tile([128, HW], BF16)
            nc.sync.dma_start(pm[:], pmat_d[:])
            idn = const.tile([128, 128], BF16)
            nc.sync.dma_start(idn[:], ident_d[:])

            st = [dict() for _ in range(BPC)]

            def load(b):
                s = st[b]
                s["q"] = qpool.tile([128, HW], BF16, tag="q", name="q")
                nc.sync.dma_start(s["q"][:], qmat_d[b])
                s["fa"] = fpool.tile([128, NT, C], BF16, tag="fa", name="fa")
                for tj in range(NT):
                    nc.sync.dma_start(
                        s["fa"][:, tj, :], fsrc_d[b, tj * 128 : (tj + 1) * 128, :]
                    )
                s["ea"] = epool.tile([128, NT, HW], BF16, tag="ea", name="ea")
                s["ms"] = stat.tile([128, NT], F32, tag="ms", name="ms")
                s["ms5"] = stat.tile([128, NT], F32, tag="ms5", name="ms5")
                s["s1"] = stat.tile([128, NT], F32, tag="s1", name="s1")
                s["r1"] = stat.tile([128, NT], F32, tag="r1", name="r1")
                s["dga"] = dgpool.tile([128, NT, 128], BF16, tag="dga", name="dga")
                s["e2"] = e2pool.tile([128, NT, HW], BF16, tag="e2", name="e2")
                s["s2"] = stat.tile([128, NT], F32, tag="s2", name="s2")
                s["r2"] = stat.tile([128, NT], F32, tag="r2", name="r2")

            def stage1(b, ti):
                # S = P^T Q as one K=6 (hi+lo) matmul pair, 4-way row-tiled:
                # tile group g = 2*(parity of sp buf) + nh -> 4 distinct PSUM
                # banks in flight across two consecutive sp tiles.
                s = st[b]
                k = b * NT + ti
                sp = spspool.tile([128, HW], F32, tag="sp")
                for nh in range(2):
                    g = 2 * (k % 2) + nh
                    nc.tensor.matmul(
                        sp[:, nh * 512 : (nh + 1) * 512],
                        pm[32 * g : 32 * g + 6, ti * 128 : (ti + 1) * 128],
                        s["q"][32 * g : 32 * g + 6, nh * 512 : (nh + 1) * 512],
                        start=True,
                        stop=True,
                        tile_position=(32 * g, 0),
                    )
                # row max of |S| straight off PSUM (negated for the exp bias)
                nc.vector.reduce_max(
                    s["ms"][:, ti : ti + 1], sp[:], axis=AX.X,
                    apply_absolute_value=True, negate=True,
                )
                # bias = -5*max|S| on GpSimd
                nc.gpsimd.tensor_scalar_mul(
                    s["ms5"][:, ti : ti + 1], s["ms"][:, ti : ti + 1], 5.0
                )
                # zt = |5S| (ACT); runs concurrently with the PSUM row-max
                zt = zpool.tile([128, HW], F32)
                nc.scalar.activation(zt[:], sp[:], AF.Abs, scale=5.0)
                # e = exp(5|S| - 5*max|S|), s1 = row sums
                nc.scalar.activation(
                    s["ea"][:, ti, :],
                    zt[:],
                    AF.Exp,
                    bias=s["ms5"][:, ti : ti + 1],
                    accum_out=s["s1"][:, ti : ti + 1],
                )

            def recip_dga(b, half):
                # batched r1 = 1/s1 for 4 tiles, then diag(r1) prep on GpSimd
                s = st[b]
                lo = half * 4
                nc.vector.reciprocal(
                    s["r1"][:, lo : lo + 4], s["s1"][:, lo : lo + 4]
                )
                for ti in range(lo, lo + 4):
                    nc.gpsimd.tensor_scalar_mul(
                        s["dga"][:, ti, :], idn[:], s["r1"][:, ti : ti + 1]
                    )

            def stage2(b, tj):
                # "transpose" via real matmul: PT[j,i'] = sum_i e[i,j]*dg[i,i']
                # = e[i',j]/s1[i'];  E2 = exp(-p) with column sums; fold 1/s2
                # into the f rows (GpSimd).
                s = st[b]
                tp = tppool.tile([128, HW], F32, tag="tp")
                # alternate PSUM banks between consecutive writes so the
                # bank-overlap tracker doesn't serialize back-to-back matmuls
                for ti in (0, 4, 1, 5, 2, 6, 3, 7):
                    nc.tensor.matmul(
                        tp[:, ti * 128 : (ti + 1) * 128],
                        s["ea"][:, ti, tj * 128 : (tj + 1) * 128],
                        s["dga"][:, ti, :],
                        start=True,
                        stop=True,
                    )
                nc.scalar.activation(
                    s["e2"][:, tj, :],
                    tp[:],
                    AF.Exp,
                    scale=-1.0,
                    accum_out=s["s2"][:, tj : tj + 1],
                )
                nc.vector.reciprocal(
                    s["r2"][:, tj : tj + 1], s["s2"][:, tj : tj + 1]
                )
                nc.gpsimd.tensor_scalar_mul(
                    s["fa"][:, tj, :], s["fa"][:, tj, :], s["r2"][:, tj : tj + 1]
                )

            def ride_alloc(b):
                s = st[b]
                s["gacc"] = [
                    gpool.tile([128, C], F32, tag=f"g{ig}", name=f"gacc{ig}")
                    for ig in range(N_RIDE)
                ]

            def ride_gemm(b, tj):
                # accumulate the first N_RIDE i-tiles as e2/fw j-tiles appear
                s = st[b]
                for ig in range(N_RIDE):
                    nc.tensor.matmul(
                        s["gacc"][ig][:],
                        s["e2"][:, tj, ig * 128 : (ig + 1) * 128],
                        s["fa"][:, tj, :],
                        start=(tj == 0),
                        stop=(tj == NT - 1),
                    )

            def ride_evict(b, on_act):
                s = st[b]
                for ig in range(N_RIDE):
                    ost = opool.tile([128, C], F32, tag=f"os{ig}")
                    if on_act:
                        nc.scalar.copy(ost[:], s["gacc"][ig][:])
                    else:
                        nc.vector.tensor_copy(ost[:], s["gacc"][ig][:])
                    nc.sync.dma_start(
                        out_d[b, ig * 128 : (ig + 1) * 128, :], ost[:]
                    )

            def post_gemm(b, pair, on_act):
                # i-tile pairs (2,3),(4,5),(6,7): two i-tiles per 2-bank PSUM
                # slot reusing the (now free) stage-1 pool; one evict + DMA.
                s = st[b]
                op_ = spspool.tile([128, 2, C], F32, tag="sp")
                for half in range(2):
                    ti = N_RIDE + 2 * pair + half
                    for tj in range(NT):
                        nc.tensor.matmul(
                            op_[:, half, :],
                            s["e2"][:, tj, ti * 128 : (ti + 1) * 128],
                            s["fa"][:, tj, :],
                            start=(tj == 0),
                            stop=(tj == NT - 1),
                        )
                ost = opool.tile([128, 2, C], F32, tag="op")
                if on_act:
                    nc.scalar.copy(ost[:], op_[:])
                else:
                    nc.vector.tensor_copy(ost[:], op_[:])
                lo = (N_RIDE + 2 * pair) * 128
                nc.sync.dma_start(
                    out_d[b, lo : lo + 256, :].rearrange("(t p) c -> p t c", p=128),
                    ost[:],
                )

            # ---- schedule ----
            load(0)
            load(1)
            # phase A: stage-1 of batch 0
            for ti in range(NT):
                stage1(0, ti)
                if ti == 3:
                    recip_dga(0, 0)
            recip_dga(0, 1)
            # phase B: stage-1 of batch 1 interleaved with stage-2 + riding
            # GEMM of batch 0 (PE order alternates so sp-buffer stalls don't
            # block transposes)
            ride_alloc(0)
            stage1(1, 0)
            stage1(1, 1)
            for k in range(NT):
                stage2(0, k)
                if k >= 1:
                    ride_gemm(0, k - 1)
                if k < NT - 2:
                    stage1(1, k + 2)
                if k == 1:
                    recip_dga(1, 0)
            recip_dga(1, 1)
            ride_gemm(0, NT - 1)
            ride_evict(0, on_act=False)
            # phase C: stage-2 + riding GEMM of batch 1, post GEMM of batch 0
            ride_alloc(1)
            for k in range(NT):
                stage2(1, k)
                if k >= 1:
                    ride_gemm(1, k - 1)
                if k % 3 == 2:
                    post_gemm(0, k // 3, on_act=False)
            ride_gemm(1, NT - 1)
            post_gemm(0, 2, on_act=False)
            # phase D: tail — ACT is idle, use it for evictions
            ride_evict(1, on_act=True)
            for pair in range(3):
                post_gemm(1, pair, on_act=True)
    nc.compile()
    return nc


_NC = None


def _get_nc():
    global _NC
    if _NC is None:
        _NC = _build_nc()
    return _NC


# ---------------------------------------------------------------- execution
def _run(inputs, trace=False):
    f_src = np.asarray(inputs["f_src"], np.float32)
    Q, P = _line_coeffs(inputs["K1"], inputs["K2"], inputs["R"], inputs["t"])

    fsrcT = np.ascontiguousarray(
        f_src.reshape(B, C, HW).transpose(0, 2, 1)
    ).astype(ml_dtypes.bfloat16)
    ident = np.eye(128, dtype=np.float32).astype(ml_dtypes.bfloat16)

    q_hi = Q.astype(ml_dtypes.bfloat16)
    q_lo = (Q - q_hi.astype(np.float32)).astype(ml_dtypes.bfloat16)
    # K=6 stack [hi; lo], replicated at partition offsets 0/32/64/96
    q6 = np.concatenate([q_hi, q_lo], axis=1)  # (B, 6, HW) bf16
    q_rep = np.zeros((B, 128, HW), dtype=ml_dtypes.bfloat16)
    p_rep = np.zeros((128, HW), dtype=ml_dtypes.bfloat16)
    p6 = np.concatenate([P, P], axis=0).astype(ml_dtypes.bfloat16)  # (6, HW)
    for g in range(4):
        q_rep[:, 32 * g : 32 * g + 6, :] = q6
        p_rep[32 * g : 32 * g + 6, :] = p6

    in_maps = []
    for core in range(NCORES):
        lo = core * BPC
        hi = lo + BPC
        in_maps.append(
            {
                "pmat": p_rep,
                "qmat": np.ascontiguousarray(q_rep[lo:hi]),
                "fsrc": np.ascontiguousarray(fsrcT[lo:hi]),
                "ident": ident,
            }
        )

    nc = _get_nc()
    res = run_bass_kernel_spmd(nc, in_maps, list(range(NCORES)), trace=trace)
    out_flat = np.concatenate(
        [res.results[i]["out"] for i in range(NCORES)], axis=0
    )  # (B, HW, C)
    out = np.ascontiguousarray(out_flat).reshape(B, C, H, W)
    return out, res


def kernel(**inputs):
    out, _ = _run(inputs, trace=False)
    return out


# revision 7
# speedup vs baseline: 2.3404x; 2.3404x over previous
"""Epipolar attention kernel for Trainium2 (8 NeuronCores, batch-parallel).

Math notes (derived from the reference):
  - f_tar is dead code: the output only depends on f_src / K1 / K2 / R / t.
  - With x0=0, x1=W the distance field factorizes rank-3:
        d[b,i,j] = |px_i*alpha[b,j] + py_i*beta[b,j] + gamma[b,j]|
    where alpha = dy/L, beta = -dx/L, gamma = y0*dx/L, L = sqrt(dx^2+dy^2).
  - softmax_j(5*(d-0.1)) == softmax_j(5*d)           (shift invariance)
  - softmax_i(1 - p)     == softmax_i(-p), and p in (0,1] means exp(-p) needs
    no max subtraction.
The 3x3 SVD / inverse chain (B=16) plus the rank-3 coefficient prep is O(B*HW)
host work; all O(B*HW^2) work runs on the NeuronCores.

v2 performance structure:
  - Stage-1 S = P^T Q runs as K=6 matmuls (hi+lo bf16 rows merged) with 4-way
    PE row tiling (tile_position) so 4 matmuls stream concurrently.
  - |S| row-max comes straight off PSUM via reduce_max(apply_absolute_value);
    the elementwise |S| pass is split between DVE (scalar_tensor_tensor) and
    ACT (Abs) to balance engine load; exp folds the x5 scale.
  - diag(1/s1) prep, the x5 bias scale, and the fw = fa/s2 row scaling run on
    the otherwise-idle GpSimd engine.
  - The output GEMM partially rides along with stage-2: 2 of 8 i-tiles
    accumulate in dedicated PSUM banks as each e2 j-tile is produced, the
    remaining 6 run from SBUF afterwards, overlapping the next batch.
"""

import numpy as np
import ml_dtypes

import concourse.bass as bass
import concourse.bacc as bacc
import concourse.tile as tile
import concourse.mybir as mybir
from concourse.bass_utils import run_bass_kernel_spmd

B, C, H, W = 16, 512, 32, 32
HW = H * W          # 1024
NCORES = 8
BPC = B // NCORES   # batches per core
NT = HW // 128      # 128-row tiles per HW dim
F32 = mybir.dt.float32
BF16 = mybir.dt.bfloat16
AF = mybir.ActivationFunctionType
AX = mybir.AxisListType
ALU = mybir.AluOpType

# i-tiles that ride along with stage-2 (accumulate per produced e2 j-tile)
N_RIDE = 2


# ---------------------------------------------------------------- host math
def _line_coeffs(K1, K2, R, t):
    """Float32 numpy mirror of the reference's per-batch line geometry.

    Returns Q (B, 3, HW) with rows [alpha, beta, gamma] and P (3, HW) with
    rows [px, py, 1].
    """
    K1 = np.asarray(K1, np.float32)
    K2 = np.asarray(K2, np.float32)
    R = np.asarray(R, np.float32)
    t = np.asarray(t, np.float32)

    z = np.zeros_like(t[:, 0])
    tx, ty, tz = t[:, 0], t[:, 1], t[:, 2]
    skew = np.stack(
        [
            np.stack([z, -tz, ty], axis=-1),
            np.stack([tz, z, -tx], axis=-1),
            np.stack([-ty, tx, z], axis=-1),
        ],
        axis=1,
    )
    E = skew @ R
    U, S, Vt = np.linalg.svd(E)
    S = S * np.array([1.0, 1.0, 0.0], dtype=S.dtype)
    E = U @ (S[:, :, None] * Vt)
    Fm = np.linalg.inv(np.swapaxes(K2, 1, 2)) @ E @ np.linalg.inv(K1)
    Fm = Fm.astype(np.float32)

    ix, iy = np.meshgrid(
        np.arange(H, dtype=np.float32), np.arange(W, dtype=np.float32), indexing="ij"
    )
    px = ix.reshape(-1)
    py = iy.reshape(-1)
    idx = np.stack([px, py, np.ones_like(px)], axis=0)  # (3, HW)

    lines = Fm @ idx[None]  # (B, 3, HW)
    a, b, c = lines[:, 0], lines[:, 1], lines[:, 2]
    x0 = np.zeros_like(a)
    y0 = -c / b
    x1 = np.full_like(a, float(W))
    y1 = -(c + a * float(W)) / b
    dx = x0 - x1
    dy = y0 - y1
    L = np.sqrt(dx * dx + dy * dy)

    alpha = dy / L
    beta = -dx / L
    gamma = (y0 * dx) / L
    Q = np.stack([alpha, beta, gamma], axis=1).astype(np.float32)  # (B, 3, HW)
    P = idx.astype(np.float32)
    return Q, P


# ---------------------------------------------------------------- device IR
def _build_nc():
    nc = bacc.Bacc("TRN2", target_bir_lowering=False, debug=False)

    # P6/Q6 carry [hi; lo] bf16 splits stacked to K=6, replicated at partition
    # offsets 0/32/64/96 so 4-way PE row tiling can run 4 matmuls at once.
    pmat_d = nc.dram_tensor("pmat", [128, HW], BF16, kind="ExternalInput")
    qmat_d = nc.dram_tensor("qmat", [BPC, 128, HW], BF16, kind="ExternalInput")
    fsrc_d = nc.dram_tensor("fsrc", [BPC, HW, C], BF16, kind="ExternalInput")
    ident_d = nc.dram_tensor("ident", [128, 128], BF16, kind="ExternalInput")
    out_d = nc.dram_tensor("out", [BPC, HW, C], F32, kind="ExternalOutput")

    with tile.TileContext(nc) as tc:
        with (
            tc.tile_pool(name="const", bufs=1) as const,
            tc.tile_pool(name="q", bufs=2) as qpool,
            tc.tile_pool(name="f", bufs=2) as fpool,
            tc.tile_pool(name="z", bufs=3) as zpool,
            tc.tile_pool(name="e", bufs=2) as epool,
            tc.tile_pool(name="dg", bufs=2) as dgpool,
            tc.tile_pool(name="e2", bufs=2) as e2pool,
            tc.tile_pool(name="stat", bufs=2) as stat,
            tc.tile_pool(name="o", bufs=4) as opool,
            tc.tile_pool(name="sps", bufs=2, space="PSUM") as spspool,
            tc.tile_pool(name="tp", bufs=1, space="PSUM") as tppool,
            tc.tile_pool(name="g", bufs=1, space="PSUM") as gpool,
        ):
            pm = const.tile([128, HW], BF16)
            nc.sync.dma_start(pm[:], pmat_d[:])
            idn = const.tile([128, 128], BF16)
            nc.sync.dma_start(idn[:], ident_d[:])

            st = [dict() for _ in range(BPC)]

            def load(b):
                s = st[b]
                s["q"] = qpool.tile([128, HW], BF16, tag="q", name="q")
                nc.sync.dma_start(s["q"][:], qmat_d[b])
                s["fa"] = fpool.tile([128, NT, C], BF16, tag="fa", name="fa")
                for tj in range(NT):
                    nc.sync.dma_start(
                        s["fa"][:, tj, :], fsrc_d[b, tj * 128 : (tj + 1) * 128, :]
                    )
                s["ea"] = epool.tile([128, NT, HW], BF16, tag="ea", name="ea")
                s["ms"] = stat.tile([128, NT], F32, tag="ms", name="ms")
                s["ms5"] = stat.tile([128, NT], F32, tag="ms5", name="ms5")
                s["s1"] = stat.tile([128, NT], F32, tag="s1", name="s1")
                s["r1"] = stat.tile([128, NT], F32, tag="r1", name="r1")
                s["dga"] = dgpool.tile([128, NT, 128], BF16, tag="dga", name="dga")
                s["e2"] = e2pool.tile([128, NT, HW], BF16, tag="e2", name="e2")
                s["s2"] = stat.tile([128, NT], F32, tag="s2", name="s2")
                s["r2"] = stat.tile([128, NT], F32, tag="r2", name="r2")

            def stage1(b, ti):
                # S = P^T Q as one K=6 (hi+lo) matmul pair, 4-way row-tiled:
                # tile group g = 2*(parity of sp buf) + nh -> 4 distinct PSUM
                # banks in flight across two consecutive sp tiles.
                s = st[b]
                k = b * NT + ti
                sp = spspool.tile([128, HW], F32, tag="sp")
                for nh in range(2):
                    g = 2 * (k % 2) + nh
                    nc.tensor.matmul(
                        sp[:, nh * 512 : (nh + 1) * 512],
                        pm[32 * g : 32 * g + 6, ti * 128 : (ti + 1) * 128],
                        s["q"][32 * g : 32 * g + 6, nh * 512 : (nh + 1) * 512],
                        start=True,
                        stop=True,
                        tile_position=(32 * g, 0),
                    )
                # row max of |S| straight off PSUM (negated for the exp bias)
                nc.vector.reduce_max(
                    s["ms"][:, ti : ti + 1], sp[:], axis=AX.X,
                    apply_absolute_value=True, negate=True,
                )
                # bias = -5*max|S| on GpSimd
                nc.vector.tensor_scalar_mul(
                    s["ms5"][:, ti : ti + 1], s["ms"][:, ti : ti + 1], 5.0
                )
                # zt = |5S| (ACT); runs concurrently with the PSUM row-max
                zt = zpool.tile([128, HW], F32)
                nc.scalar.activation(zt[:], sp[:], AF.Abs, scale=5.0)
                # e = exp(5|S| - 5*max|S|), s1 = row sums
                nc.scalar.activation(
                    s["ea"][:, ti, :],
                    zt[:],
                    AF.Exp,
                    bias=s["ms5"][:, ti : ti + 1],
                    accum_out=s["s1"][:, ti : ti + 1],
                )

            def recip_dga(b, half):
                # batched r1 = 1/s1 for 4 tiles, then diag(r1) prep on GpSimd
                s = st[b]
                lo = half * 4
                nc.vector.reciprocal(
                    s["r1"][:, lo : lo + 4], s["s1"][:, lo : lo + 4]
                )
                for ti in range(lo, lo + 4):
                    nc.vector.tensor_scalar_mul(
                        s["dga"][:, ti, :], idn[:], s["r1"][:, ti : ti + 1]
                    )

            def stage2(b, tj):
                # "transpose" via real matmul: PT[j,i'] = sum_i e[i,j]*dg[i,i']
                # = e[i',j]/s1[i'];  E2 = exp(-p) with column sums; fold 1/s2
                # into the f rows (GpSimd).
                s = st[b]
                tp = tppool.tile([128, HW], F32, tag="tp")
                # alternate PSUM banks between consecutive writes so the
                # bank-overlap tracker doesn't serialize back-to-back matmuls
                for ti in (0, 4, 1, 5, 2, 6, 3, 7):
                    nc.tensor.matmul(
                        tp[:, ti * 128 : (ti + 1) * 128],
                        s["ea"][:, ti, tj * 128 : (tj + 1) * 128],
                        s["dga"][:, ti, :],
                        start=True,
                        stop=True,
                    )
                nc.scalar.activation(
                    s["e2"][:, tj, :],
                    tp[:],
                    AF.Exp,
                    scale=-1.0,
                    accum_out=s["s2"][:, tj : tj + 1],
                )
                nc.vector.reciprocal(
                    s["r2"][:, tj : tj + 1], s["s2"][:, tj : tj + 1]
                )
                nc.vector.tensor_scalar_mul(
                    s["fa"][:, tj, :], s["fa"][:, tj, :], s["r2"][:, tj : tj + 1]
                )

            def ride_alloc(b):
                s = st[b]
                s["gacc"] = [
                    gpool.tile([128, C], F32, tag=f"g{ig}", name=f"gacc{ig}")
                    for ig in range(N_RIDE)
                ]

            def ride_gemm(b, tj):
                # accumulate the first N_RIDE i-tiles as e2/fw j-tiles appear
                s = st[b]
                for ig in range(N_RIDE):
                    nc.tensor.matmul(
                        s["gacc"][ig][:],
                        s["e2"][:, tj, ig * 128 : (ig + 1) * 128],
                        s["fa"][:, tj, :],
                        start=(tj == 0),
                        stop=(tj == NT - 1),
                    )

            def ride_evict(b, on_act):
                s = st[b]
                for ig in range(N_RIDE):
                    ost = opool.tile([128, C], F32, tag=f"os{ig}")
                    if on_act:
                        nc.scalar.copy(ost[:], s["gacc"][ig][:])
                    else:
                        nc.vector.tensor_copy(ost[:], s["gacc"][ig][:])
                    nc.sync.dma_start(
                        out_d[b, ig * 128 : (ig + 1) * 128, :], ost[:]
                    )

            def post_gemm(b, pair, on_act):
                # i-tile pairs (2,3),(4,5),(6,7): two i-tiles per 2-bank PSUM
                # slot reusing the (now free) stage-1 pool; one evict + DMA.
                s = st[b]
                op_ = spspool.tile([128, 2, C], F32, tag="sp")
                for half in range(2):
                    ti = N_RIDE + 2 * pair + half
                    for tj in range(NT):
                        nc.tensor.matmul(
                            op_[:, half, :],
                            s["e2"][:, tj, ti * 128 : (ti + 1) * 128],
                            s["fa"][:, tj, :],
                            start=(tj == 0),
                            stop=(tj == NT - 1),
                        )
                ost = opool.tile([128, 2, C], F32, tag="op")
                if on_act:
                    nc.scalar.copy(ost[:], op_[:])
                else:
                    nc.vector.tensor_copy(ost[:], op_[:])
                lo = (N_RIDE + 2 * pair) * 128
                nc.sync.dma_start(
                    out_d[b, lo : lo + 256, :].rearrange("(t p) c -> p t c", p=128),
                    ost[:],
                )

            # ---- schedule ----
            load(0)
            load(1)
            # phase A: stage-1 of batch 0
            for ti in range(NT):
                stage1(0, ti)
                if ti == 3:
                    recip_dga(0, 0)
            recip_dga(0, 1)
            # phase B: stage-1 of batch 1 interleaved with stage-2 + riding
            # GEMM of batch 0 (PE order alternates so sp-buffer stalls don't
            # block transposes)
            ride_alloc(0)
            stage1(1, 0)
            stage1(1, 1)
            for k in range(NT):
                stage2(0, k)
                if k >= 1:
                    ride_gemm(0, k - 1)
                if k < NT - 2:
                    stage1(1, k + 2)
                if k == 1:
                    recip_dga(1, 0)
            recip_dga(1, 1)
            ride_gemm(0, NT - 1)
            ride_evict(0, on_act=False)
            # phase C: stage-2 + riding GEMM of batch 1, post GEMM of batch 0
            ride_alloc(1)
            for k in range(NT):
                stage2(1, k)
                if k >= 1:
                    ride_gemm(1, k - 1)
                if k % 3 == 2:
                    post_gemm(0, k // 3, on_act=False)
            ride_gemm(1, NT - 1)
            post_gemm(0, 2, on_act=False)
            # phase D: tail — ACT is idle, use it for evictions
            ride_evict(1, on_act=True)
            for pair in range(3):
                post_gemm(1, pair, on_act=True)
    nc.compile()
    return nc


_NC = None


def _get_nc():
    global _NC
    if _NC is None:
        _NC = _build_nc()
    return _NC


# ---------------------------------------------------------------- execution
def _run(inputs, trace=False):
    f_src = np.asarray(inputs["f_src"], np.float32)
    Q, P = _line_coeffs(inputs["K1"], inputs["K2"], inputs["R"], inputs["t"])

    fsrcT = np.ascontiguousarray(
        f_src.reshape(B, C, HW).transpose(0, 2, 1)
    ).astype(ml_dtypes.bfloat16)
    ident = np.eye(128, dtype=np.float32).astype(ml_dtypes.bfloat16)

    q_hi = Q.astype(ml_dtypes.bfloat16)
    q_lo = (Q - q_hi.astype(np.float32)).astype(ml_dtypes.bfloat16)
    # K=6 stack [hi; lo], replicated at partition offsets 0/32/64/96
    q6 = np.concatenate([q_hi, q_lo], axis=1)  # (B, 6, HW) bf16
    q_rep = np.zeros((B, 128, HW), dtype=ml_dtypes.bfloat16)
    p_rep = np.zeros((128, HW), dtype=ml_dtypes.bfloat16)
    p6 = np.concatenate([P, P], axis=0).astype(ml_dtypes.bfloat16)  # (6, HW)
    for g in range(4):
        q_rep[:, 32 * g : 32 * g + 6, :] = q6
        p_rep[32 * g : 32 * g + 6, :] = p6

    in_maps = []
    for core in range(NCORES):
        lo = core * BPC
        hi = lo + BPC
        in_maps.append(
            {
                "pmat": p_rep,
                "qmat": np.ascontiguousarray(q_rep[lo:hi]),
                "fsrc": np.ascontiguousarray(fsrcT[lo:hi]),
                "ident": ident,
            }
        )

    nc = _get_nc()
    res = run_bass_kernel_spmd(nc, in_maps, list(range(NCORES)), trace=trace)
    out_flat = np.concatenate(
        [res.results[i]["out"] for i in range(NCORES)], axis=0
    )  # (B, HW, C)
    out = np.ascontiguousarray(out_flat).reshape(B, C, H, W)
    return out, res


def kernel(**inputs):
    out, _ = _run(inputs, trace=False)
    return out
